# revision 1
# baseline (speedup 1.0000x reference)
"""Multi-head attention (nn_GroupQueryAttention_163208757512) on 8 TRN2 cores.

Problem: B=2, S=2048, D=1024, H=16 heads, DK=64. f32.
    q = Q @ Wq.T + bq  (per head)   k, v likewise
    out = softmax(q k^T / 8) v  -> concat heads -> @ Wo.T + bo

Sharding: core c handles batch b=c//4 and head group g=c%4 (4 heads,
feature slice hs = 256*g : 256*g+256). Data parallel on B, tensor
parallel on heads; the output projection yields per-core partials that
the host sums (replaces the all-reduce).

Device-side design (all PE matmuls in float32r: 1 cycle/row, ~1e-4 rounding):
  - host supplies X^T = {Q,K,V}[b].T so projections write q^T,k^T in
    [dh, s] layout directly; v is produced in natural [s, dh] layout.
  - scores are computed pre-transposed: S^T[sk, sq] = K_h Q_h^T, with
    two heads row-packed into the 128x128 PE array (K=64 each).
  - exp runs on ScalarE straight out of 2-bank PSUM with scale=1/8.
  - PV uses the stationary [v_h | 1] trick (M=65): column 64 accumulates
    the softmax denominators for free.
  - 1/denom is broadcast across partitions with a K=1 rank-1 matmul
    (ones^T x recip) and applied on VectorE during PSUM evacuation,
    which also adds bv. Result x_norm^T is the out-proj lhsT.
  - bo enters via a rank-1 ones x bo product added during output
    evacuation (only on the g==0 core of each batch).

Constraint discovered on this toolchain: walrus allows ONE sync-wait per
instruction, so a post-pass (split_waits) chains excess waits onto NoOps.
Accumulation groups must keep one lhsT base partition (HW fault otherwise).
"""

import os
import numpy as np
from contextlib import ExitStack

import concourse.bass as bass
import concourse.mybir as mybir
import concourse.tile as tile
from concourse.bass import ds, ts
from concourse.bass_utils import run_bass_kernel_spmd

F32 = mybir.dt.float32
F32R = mybir.dt.float32r
AF = mybir.ActivationFunctionType
ALU = mybir.AluOpType

B, S, D, H = 2, 2048, 1024, 16
DK = D // H            # 64
NCORES = 8
GROUPS = 4             # head groups per batch
DH = D // GROUPS       # 256 feature cols per core
P = 128
KT = D // P            # 8 contraction tiles for projections
ST = S // P            # 16 s-tiles
CH = 4                 # s-chunks
CW = S // CH           # 512


# ---------------------------------------------------------------- wait fix
_wf_counter = [0]


def _split_waits(nc, cap=1):
    """walrus in this container accepts at most one sync-wait command per
    instruction; chain the rest onto same-engine NoOps placed just before."""
    for fn in nc.m.functions:
        for bb in fn.blocks:
            out, changed = [], False
            for inst in bb.instructions:
                si = inst.sync_info
                waits = list(si.on_wait) if (si is not None and si.on_wait) else []
                if len(waits) > cap:
                    changed = True
                    keep = waits[-cap:]
                    for i in range(0, len(waits) - cap, cap):
                        _wf_counter[0] += 1
                        out.append(mybir.InstNoOp(
                            name=f"waitfix_{_wf_counter[0]}",
                            sync_info=mybir.SyncInfo(
                                on_wait=waits[i:i + cap], on_update=[]),
                            engine=inst.engine,
                            bass_nofuse=True,
                        ))
                    inst.sync_info = mybir.SyncInfo(
                        on_wait=keep,
                        on_update=list(si.on_update) if si else [])
                out.append(inst)
            if changed:
                bb.instructions = out
    return nc


# ---------------------------------------------------------------- program
def build_program(apply_waitfix=True):
    nc = bass.Bass()

    xqt = nc.dram_tensor("xqt", [D, S], F32R, kind="ExternalInput")
    xkt = nc.dram_tensor("xkt", [D, S], F32R, kind="ExternalInput")
    xvt = nc.dram_tensor("xvt", [D, S], F32R, kind="ExternalInput")
    wqt = nc.dram_tensor("wqt", [D, DH], F32R, kind="ExternalInput")
    wkt = nc.dram_tensor("wkt", [D, DH], F32R, kind="ExternalInput")
    wvt = nc.dram_tensor("wvt", [D, DH], F32R, kind="ExternalInput")
    wot = nc.dram_tensor("wot", [DH, D], F32R, kind="ExternalInput")
    bq2 = nc.dram_tensor("bq2", [P, 2], F32, kind="ExternalInput")
    bk2 = nc.dram_tensor("bk2", [P, 2], F32, kind="ExternalInput")
    bv2 = nc.dram_tensor("bv2", [P, 2], F32, kind="ExternalInput")
    bo_eff = nc.dram_tensor("bo_eff", [1, D], F32R, kind="ExternalInput")
    onesd = nc.dram_tensor("onesd", [1, P], F32R, kind="ExternalInput")
    onespv = nc.dram_tensor("onespv", [P, ST, GROUPS, 1], F32R,
                            kind="ExternalInput")
    y = nc.dram_tensor("y", [S, D], F32, kind="ExternalOutput")

    xqt_r = xqt.rearrange("(kt p) s -> kt p s", p=P)
    xkt_r = xkt.rearrange("(kt p) s -> kt p s", p=P)
    xvt_r = xvt.rearrange("(kt p) s -> kt p s", p=P)
    y_r = y.rearrange("(st p) d -> st p d", p=P)

    with tile.TileContext(nc) as tc:
      with ExitStack() as ctx:
        # ---- persistent SBUF ----
        wp = ctx.enter_context(tc.tile_pool(name="wp", bufs=1))
        wq_sb = wp.tile([P, KT, DH], F32R, tag="wq")
        wk_sb = wp.tile([P, KT, DH], F32R, tag="wk")
        wv_sb = wp.tile([P, KT, DH], F32R, tag="wv")
        wo_sb = wp.tile([P, 2, D], F32R, tag="wo")
        bq_sb = wp.tile([P, 2], F32, tag="bq")
        bk_sb = wp.tile([P, 2], F32, tag="bk")
        bv_sb = wp.tile([P, 2], F32, tag="bv")
        ones1 = wp.tile([1, P], F32R, tag="ones1")
        bo_sb = wp.tile([1, D], F32R, tag="bo")
        borep_sb = wp.tile([P, D], F32, tag="borep")

        qt_sb = wp.tile([P, 2, S], F32R, tag="qt")
        kt_sb = wp.tile([P, 2, S], F32R, tag="kt")
        pvw_sb = wp.tile([P, ST, GROUPS, DK + 1], F32R, tag="pvw")
        xn_sb = wp.tile([P, 2, S], F32R, tag="xn")

        nc.sync.dma_start(wq_sb[:], wqt.rearrange("(kt p) m -> p kt m", p=P))
        nc.sync.dma_start(wk_sb[:], wkt.rearrange("(kt p) m -> p kt m", p=P))
        nc.sync.dma_start(wv_sb[:], wvt.rearrange("(kt p) m -> p kt m", p=P))
        nc.sync.dma_start(wo_sb[:], wot.rearrange("(p2 p) d -> p p2 d", p=P))
        nc.sync.dma_start(bq_sb[:], bq2[:])
        nc.sync.dma_start(bk_sb[:], bk2[:])
        nc.sync.dma_start(bv_sb[:], bv2[:])
        nc.sync.dma_start(ones1[:], onesd[:])
        nc.sync.dma_start(bo_sb[:], bo_eff[:])
        nc.sync.dma_start(pvw_sb[:, :, :, DK:DK + 1], onespv[:])

        with nc.allow_low_precision(reason="float32r is fp32 rounded ~1e-4"):
          # ---------------- phase B: projections ----------------
          with (
              tc.tile_pool(name="xs", bufs=10) as xs,
              tc.tile_pool(name="pp", bufs=3, space="PSUM") as proj_ps,
              tc.tile_pool(name="vp", bufs=2, space="PSUM") as vproj_ps,
          ):
            # bo broadcast (rank-1) for the output stage
            for oc in range(2):
                bp = proj_ps.tile([P, CW], F32, tag="p")
                nc.tensor.matmul(bp[:], ones1[:], bo_sb[:, ds(CW * oc, CW)],
                                 start=True, stop=True)
                nc.vector.tensor_copy(borep_sb[:, ds(CW * oc, CW)], bp[:])

            for c in range(CH):
                csl = ds(CW * c, CW)
                qx, kx, vx = [], [], []
                for kt in range(KT):
                    t = xs.tile([P, CW], F32R, tag="xq")
                    nc.sync.dma_start(t[:], xqt_r[kt, :, csl])
                    qx.append(t)
                for kt in range(KT):
                    t = xs.tile([P, CW], F32R, tag="xk")
                    nc.sync.dma_start(t[:], xkt_r[kt, :, csl])
                    kx.append(t)
                for kt in range(KT):
                    t = xs.tile([P, CW], F32R, tag="xv")
                    nc.sync.dma_start(t[:], xvt_r[kt, :, csl])
                    vx.append(t)

                for p in range(2):
                    pp = proj_ps.tile([P, CW], F32, tag="p")
                    for kt in range(KT):
                        nc.tensor.matmul(pp[:], wq_sb[:, kt, ds(P * p, P)],
                                         qx[kt][:],
                                         start=(kt == 0), stop=(kt == KT - 1))
                    nc.vector.tensor_scalar_add(qt_sb[:, p, csl], pp[:],
                                                bq_sb[:, p:p + 1])
                for p in range(2):
                    pp = proj_ps.tile([P, CW], F32, tag="p")
                    for kt in range(KT):
                        nc.tensor.matmul(pp[:], wk_sb[:, kt, ds(P * p, P)],
                                         kx[kt][:],
                                         start=(kt == 0), stop=(kt == KT - 1))
                    nc.vector.tensor_scalar_add(kt_sb[:, p, csl], pp[:],
                                                bk_sb[:, p:p + 1])
                for st4 in range(4):
                    vp = vproj_ps.tile([P, DH], F32, tag="v")
                    for kt in range(KT):
                        nc.tensor.matmul(vp[:], vx[kt][:, ds(P * st4, P)],
                                         wv_sb[:, kt, :],
                                         start=(kt == 0), stop=(kt == KT - 1))
                    st = 4 * c + st4
                    nc.vector.tensor_copy(
                        pvw_sb[:, st, :, 0:DK],
                        vp[:].rearrange("p (h d) -> p h d", h=GROUPS))

          # ---------------- phase C: attention ----------------
          with (
              tc.tile_pool(name="ptp", bufs=4) as ptp,
              tc.tile_pool(name="rcps", bufs=2) as rcps,
              tc.tile_pool(name="reps", bufs=2) as repsb,
              tc.tile_pool(name="spp", bufs=2, space="PSUM") as sp_ps,
              tc.tile_pool(name="xap", bufs=2, space="PSUM") as xa_ps,
              tc.tile_pool(name="repp", bufs=2, space="PSUM") as rep_ps,
          ):
            for p in range(2):
                for c in range(CH):
                    csl = ds(CW * c, CW)
                    xaugs = [xa_ps.tile([P, CW], F32, tag="xaug",
                                        name=f"xaug_{p}_{c}_{i}")
                             for i in range(2)]
                    for grp in range(ST // 2):
                        for hh in range(2):
                            sp = sp_ps.tile([P, 2, CW], F32, tag="sp")
                            for j in range(2):
                                sk = 2 * grp + j
                                nc.tensor.matmul(
                                    sp[:, j, :],
                                    kt_sb[64 * hh:64 * hh + 64, p, ts(sk, P)],
                                    qt_sb[64 * hh:64 * hh + 64, p, csl],
                                    start=True, stop=True,
                                    tile_position=(64 * hh, 0))
                            pt = ptp.tile([P, 2, CW], F32R, tag="pt")
                            nc.scalar.activation(pt[:], sp[:], AF.Exp,
                                                 scale=0.125)
                            for j in range(2):
                                sk = 2 * grp + j
                                nc.tensor.matmul(
                                    xaugs[hh][0:DK + 1, :],
                                    pvw_sb[:, sk, 2 * p + hh, :],
                                    pt[:, j, :],
                                    start=(grp == 0 and j == 0),
                                    stop=(grp == ST // 2 - 1 and j == 1))
                    for hh in range(2):
                        rcp = rcps.tile([1, CW], F32R, tag="rcp")
                        nc.vector.reciprocal(rcp[:], xaugs[hh][DK:DK + 1, :])
                        rep = rep_ps.tile([P, CW], F32, tag="rep")
                        nc.tensor.matmul(rep[0:DK, :], ones1[:1, 0:DK],
                                         rcp[:], start=True, stop=True)
                        repc = repsb.tile([DK, CW], F32, tag="repc")
                        nc.vector.tensor_copy(repc[:], rep[0:DK, :])
                        xsl = xn_sb[64 * hh:64 * hh + 64, p, csl]
                        nc.vector.tensor_tensor(
                            xsl, xaugs[hh][0:DK, :], repc[:], ALU.mult)
                        nc.vector.tensor_scalar_add(
                            xsl, xsl,
                            bv_sb[64 * hh:64 * hh + 64, p:p + 1])

          # ---------------- phase D: output projection ----------------
          with (
              tc.tile_pool(name="ev", bufs=4) as ev,
              tc.tile_pool(name="yp", bufs=4, space="PSUM") as y_ps,
          ):
            for st in range(ST):
                for oc in range(2):
                    yp = y_ps.tile([P, CW], F32, tag="y")
                    for p2 in range(2):
                        nc.tensor.matmul(yp[:], xn_sb[:, p2, ts(st, P)],
                                         wo_sb[:, p2, ds(CW * oc, CW)],
                                         start=(p2 == 0), stop=(p2 == 1))
                    ysb = ev.tile([P, CW], F32, tag="ysb")
                    nc.vector.tensor_tensor(ysb[:], yp[:],
                                            borep_sb[:, ds(CW * oc, CW)],
                                            ALU.add)
                    nc.sync.dma_start(y_r[st, :, ds(CW * oc, CW)], ysb[:])

    if apply_waitfix:
        _split_waits(nc, cap=1)
    return nc


_program_cache = {}


def get_program():
    if "nc" not in _program_cache:
        _program_cache["nc"] = build_program()
    return _program_cache["nc"]


def make_in_maps(Q, K, V, Wq, bq, Wk, bk, Wv, bv, Wo, bo):
    Q = np.asarray(Q, dtype=np.float32)
    K = np.asarray(K, dtype=np.float32)
    V = np.asarray(V, dtype=np.float32)
    Wq = np.asarray(Wq, dtype=np.float32)
    Wk = np.asarray(Wk, dtype=np.float32)
    Wv = np.asarray(Wv, dtype=np.float32)
    Wo = np.asarray(Wo, dtype=np.float32)
    bq = np.asarray(bq, dtype=np.float32)
    bk = np.asarray(bk, dtype=np.float32)
    bv = np.asarray(bv, dtype=np.float32)
    bo = np.asarray(bo, dtype=np.float32)

    xt = {b: {
        "q": np.ascontiguousarray(Q[b].T),
        "k": np.ascontiguousarray(K[b].T),
        "v": np.ascontiguousarray(V[b].T),
    } for b in range(B)}

    ones1 = np.ones((1, P), dtype=np.float32)
    onespv = np.ones((P, ST, GROUPS, 1), dtype=np.float32)
    zero_bo = np.zeros((1, D), dtype=np.float32)
    bo_row = bo.reshape(1, D)

    in_maps = []
    for c in range(NCORES):
        b, g = divmod(c, GROUPS)
        hs = slice(DH * g, DH * (g + 1))
        in_maps.append({
            "xqt": xt[b]["q"],
            "xkt": xt[b]["k"],
            "xvt": xt[b]["v"],
            "wqt": np.ascontiguousarray(Wq[hs, :].T),
            "wkt": np.ascontiguousarray(Wk[hs, :].T),
            "wvt": np.ascontiguousarray(Wv[hs, :].T),
            "wot": np.ascontiguousarray(Wo[:, hs].T),
            "bq2": np.ascontiguousarray(bq[hs].reshape(2, P).T),
            "bk2": np.ascontiguousarray(bk[hs].reshape(2, P).T),
            "bv2": np.ascontiguousarray(bv[hs].reshape(2, P).T),
            "bo_eff": bo_row if g == 0 else zero_bo,
            "onesd": ones1,
            "onespv": onespv,
        })
    return in_maps


def combine_outputs(results):
    """results: list of 8 per-core dicts with 'y' [S, D] partials."""
    out = np.zeros((B, S, D), dtype=np.float32)
    for c, res in enumerate(results):
        b = c // GROUPS
        out[b] += res["y"]
    return out


def kernel(**inputs) -> np.ndarray:
    nc = get_program()
    in_maps = make_in_maps(**inputs)
    res = run_bass_kernel_spmd(nc, in_maps, core_ids=list(range(NCORES)))
    return combine_outputs(res.results)



# revision 16
# speedup vs baseline: 1.1761x; 1.1761x over previous
"""Multi-head attention (nn_GroupQueryAttention_163208757512) on 8 TRN2 cores.

Problem: B=2, S=2048, D=1024, H=16 heads, DK=64. f32 in/out.
    q = Q @ Wq.T + bq  (per head)   k, v likewise
    out = softmax(q k^T / 8) v -> concat heads -> @ Wo.T + bo

Sharding: core c handles batch b=c//4 and head group g=c%4 (4 heads,
feature slice hs = 256*g : 256*g+256). Data parallel on B, tensor
parallel on heads; the output projection yields per-core partials that
the host sums (replaces the all-reduce).

Device-side design (v2 — all PE matmuls in bf16, f32 PSUM accumulate):
  - host supplies X^T = {Q,K,V}[b].T pre-cast to bf16 so projections
    write q^T,k^T in [dh, s] layout directly; v in natural [s, dh].
  - bf16 matmuls stream 1 row/cycle at 2.4 GHz (fp32r was 4x slower,
    SBUF-read bound) and enable fast-weight-load.
  - scores are computed pre-transposed: S^T[sk, sq] = K_h Q_h^T, two
    heads row-packed into the 128x128 PE array (K=64 each).
  - exp runs on ScalarE straight out of 2-bank PSUM with scale=1/8,
    writing bf16 probabilities.
  - PV uses the stationary [v_h | 1] trick (M=65): column 64 accumulates
    the softmax denominators for free.
  - 1/denom on VectorE (custom DVE ops and ALU divide are rejected by
    this walrus build), broadcast across partitions with a K=1 rank-1
    f32r matmul, applied on VectorE during PSUM evacuation together
    with the bv bias add.
  - output projection is interleaved per sequence chunk right after the
    chunk's attention epilogue; bo enters via a precomputed broadcast
    row added during output evacuation (only on the g==0 core of each
    batch).

Constraint discovered on this toolchain: walrus allows ONE sync-wait per
instruction, so a post-pass (split_waits) chains excess waits onto NoOps.
Accumulation groups must keep one lhsT base partition (HW fault otherwise).
"""

import numpy as np
from contextlib import ExitStack

import ml_dtypes

import concourse.bass as bass
import concourse.mybir as mybir
import concourse.tile as tile
from concourse.bass import ds, ts
from concourse.bass_utils import run_bass_kernel_spmd

F32 = mybir.dt.float32
F32R = mybir.dt.float32r
BF16 = mybir.dt.bfloat16
AF = mybir.ActivationFunctionType
ALU = mybir.AluOpType
BF_NP = ml_dtypes.bfloat16

B, S, D, H = 2, 2048, 1024, 16
DK = D // H            # 64
NCORES = 8
GROUPS = 4             # head groups per batch
DH = D // GROUPS       # 256 feature cols per core
P = 128
KT = D // P            # 8 contraction tiles for projections
ST = S // P            # 16 s-tiles
CH = 4                 # s-chunks
CW = S // CH           # 512


# ---------------------------------------------------------------- wait fix
_wf_counter = [0]


def _split_waits(nc, cap=1):
    """walrus in this container accepts at most one sync-wait command per
    instruction; chain the rest onto same-engine NoOps placed just before."""
    for fn in nc.m.functions:
        for bb in fn.blocks:
            out, changed = [], False
            for inst in bb.instructions:
                si = inst.sync_info
                waits = list(si.on_wait) if (si is not None and si.on_wait) else []
                if len(waits) > cap:
                    changed = True
                    keep = waits[-cap:]
                    for i in range(0, len(waits) - cap, cap):
                        _wf_counter[0] += 1
                        out.append(mybir.InstNoOp(
                            name=f"waitfix_{_wf_counter[0]}",
                            sync_info=mybir.SyncInfo(
                                on_wait=waits[i:i + cap], on_update=[]),
                            engine=inst.engine,
                            bass_nofuse=True,
                        ))
                    inst.sync_info = mybir.SyncInfo(
                        on_wait=keep,
                        on_update=list(si.on_update) if si else [])
                out.append(inst)
            if changed:
                bb.instructions = out
    return nc


# ---------------------------------------------------------------- program
def build_program(apply_waitfix=True):
    nc = bass.Bass()

    xqt = nc.dram_tensor("xqt", [D, S], BF16, kind="ExternalInput")
    xkt = nc.dram_tensor("xkt", [D, S], BF16, kind="ExternalInput")
    xvt = nc.dram_tensor("xvt", [D, S], BF16, kind="ExternalInput")
    wqt = nc.dram_tensor("wqt", [D, DH], BF16, kind="ExternalInput")
    wkt = nc.dram_tensor("wkt", [D, DH], BF16, kind="ExternalInput")
    wvt = nc.dram_tensor("wvt", [D, DH], BF16, kind="ExternalInput")
    wot = nc.dram_tensor("wot", [DH, D], BF16, kind="ExternalInput")
    bq2 = nc.dram_tensor("bq2", [P, 2], F32, kind="ExternalInput")
    bk2 = nc.dram_tensor("bk2", [P, 2], F32, kind="ExternalInput")
    bv2 = nc.dram_tensor("bv2", [P, 2], F32, kind="ExternalInput")
    bo_eff = nc.dram_tensor("bo_eff", [1, D], F32R, kind="ExternalInput")
    onesd = nc.dram_tensor("onesd", [1, P], F32R, kind="ExternalInput")
    y = nc.dram_tensor("y", [S, D], F32, kind="ExternalOutput")

    xqt_r = xqt.rearrange("(kt p) s -> kt p s", p=P)
    xkt_r = xkt.rearrange("(kt p) s -> kt p s", p=P)
    xvt_r = xvt.rearrange("(kt p) s -> kt p s", p=P)
    y_r = y.rearrange("(st p) d -> st p d", p=P)

    with tile.TileContext(nc) as tc:
      with ExitStack() as ctx:
        # ---- persistent SBUF ----
        wp = ctx.enter_context(tc.tile_pool(name="wp", bufs=1))
        wq_sb = wp.tile([P, KT, DH], BF16, tag="wq")
        wk_sb = wp.tile([P, KT, DH], BF16, tag="wk")
        wv_sb = wp.tile([P, KT, DH], BF16, tag="wv")
        wo_sb = wp.tile([P, 2, D], BF16, tag="wo")
        bq_sb = wp.tile([P, 2], F32, tag="bq")
        bk_sb = wp.tile([P, 2], F32, tag="bk")
        bv_sb = wp.tile([P, 2], F32, tag="bv")
        ones1 = wp.tile([1, P], F32R, tag="ones1")
        bo_sb = wp.tile([1, D], F32R, tag="bo")
        borep_sb = wp.tile([P, D], F32, tag="borep")

        qt_sb = wp.tile([P, 2, S], BF16, tag="qt")
        kt_sb = wp.tile([P, 2, S], BF16, tag="kt")
        pvw_sb = wp.tile([P, ST, GROUPS, DK + 1], BF16, tag="pvw")
        xn_sb = wp.tile([P, 2, S], BF16, tag="xn")

        nc.sync.dma_start(wq_sb[:], wqt.rearrange("(kt p) m -> p kt m", p=P))
        nc.sync.dma_start(wk_sb[:], wkt.rearrange("(kt p) m -> p kt m", p=P))
        nc.sync.dma_start(wv_sb[:], wvt.rearrange("(kt p) m -> p kt m", p=P))
        nc.sync.dma_start(wo_sb[:], wot.rearrange("(p2 p) d -> p p2 d", p=P))
        nc.sync.dma_start(bq_sb[:], bq2[:])
        nc.sync.dma_start(bk_sb[:], bk2[:])
        nc.sync.dma_start(bv_sb[:], bv2[:])
        nc.sync.dma_start(ones1[:], onesd[:])
        nc.sync.dma_start(bo_sb[:], bo_eff[:])
        # ones column of the [v | 1] PV stationary
        nc.vector.memset(pvw_sb[:, :, :, DK:DK + 1], 1.0)

        with nc.allow_low_precision(reason="bf16 matmuls, tol is 2e-2"):
          # ---------------- phase B: projections ----------------
          with (
              tc.tile_pool(name="xs", bufs=16) as xs,
              tc.tile_pool(name="pp", bufs=3, space="PSUM") as proj_ps,
              tc.tile_pool(name="vp", bufs=2, space="PSUM") as vproj_ps,
          ):
            # bo broadcast (rank-1) for the output stage
            for oc in range(2):
                bp = proj_ps.tile([P, CW], F32, tag="p")
                nc.tensor.matmul(bp[:], ones1[:], bo_sb[:, ds(CW * oc, CW)],
                                 start=True, stop=True)
                nc.vector.tensor_copy(borep_sb[:, ds(CW * oc, CW)], bp[:])

            for c in range(CH):
                csl = ds(CW * c, CW)
                qx, kx, vx = [], [], []
                for kt in range(KT):
                    t = xs.tile([P, CW], BF16, tag="xq")
                    nc.sync.dma_start(t[:], xqt_r[kt, :, csl])
                    qx.append(t)
                for kt in range(KT):
                    t = xs.tile([P, CW], BF16, tag="xk")
                    nc.sync.dma_start(t[:], xkt_r[kt, :, csl])
                    kx.append(t)
                for kt in range(KT):
                    t = xs.tile([P, CW], BF16, tag="xv")
                    nc.sync.dma_start(t[:], xvt_r[kt, :, csl])
                    vx.append(t)

                for p in range(2):
                    pp = proj_ps.tile([P, CW], F32, tag="p")
                    for kt in range(KT):
                        nc.tensor.matmul(pp[:], wq_sb[:, kt, ds(P * p, P)],
                                         qx[kt][:],
                                         start=(kt == 0), stop=(kt == KT - 1))
                    nc.vector.tensor_scalar_add(qt_sb[:, p, csl], pp[:],
                                                bq_sb[:, p:p + 1])
                for p in range(2):
                    pp = proj_ps.tile([P, CW], F32, tag="p")
                    for kt in range(KT):
                        nc.tensor.matmul(pp[:], wk_sb[:, kt, ds(P * p, P)],
                                         kx[kt][:],
                                         start=(kt == 0), stop=(kt == KT - 1))
                    nc.vector.tensor_scalar_add(kt_sb[:, p, csl], pp[:],
                                                bk_sb[:, p:p + 1])
                for st4 in range(4):
                    vp = vproj_ps.tile([P, DH], F32, tag="v")
                    for kt in range(KT):
                        nc.tensor.matmul(vp[:], vx[kt][:, ds(P * st4, P)],
                                         wv_sb[:, kt, :],
                                         start=(kt == 0), stop=(kt == KT - 1))
                    st = 4 * c + st4
                    nc.vector.tensor_copy(
                        pvw_sb[:, st, :, 0:DK],
                        vp[:].rearrange("p (h d) -> p h d", h=GROUPS))

          # ---------- phase C: attention (+ interleaved out-proj) ----------
          with (
              tc.tile_pool(name="ptp", bufs=4) as ptp,
              tc.tile_pool(name="rcps", bufs=2) as rcps,
              tc.tile_pool(name="reps", bufs=2) as repsb,
              tc.tile_pool(name="ev", bufs=4) as ev,
              tc.tile_pool(name="spp", bufs=2, space="PSUM") as sp_ps,
              tc.tile_pool(name="xap", bufs=2, space="PSUM") as xa_ps,
              tc.tile_pool(name="mpp", bufs=2, space="PSUM") as misc_ps,
          ):
            for c in range(CH):
                csl = ds(CW * c, CW)
                for p in range(2):
                    xaugs = [xa_ps.tile([P, CW], F32, tag="xaug",
                                        name=f"xaug_{p}_{c}_{i}")
                             for i in range(2)]
                    for grp in range(ST // 2):
                        for hh in range(2):
                            sp = sp_ps.tile([P, 2, CW], F32, tag="sp")
                            for j in range(2):
                                sk = 2 * grp + j
                                nc.tensor.matmul(
                                    sp[:, j, :],
                                    kt_sb[64 * hh:64 * hh + 64, p, ts(sk, P)],
                                    qt_sb[64 * hh:64 * hh + 64, p, csl],
                                    start=True, stop=True,
                                    tile_position=(64 * hh, 0))
                            pt = ptp.tile([P, 2, CW], BF16, tag="pt")
                            nc.scalar.activation(pt[:], sp[:], AF.Exp,
                                                 scale=0.125)
                            for j in range(2):
                                sk = 2 * grp + j
                                nc.tensor.matmul(
                                    xaugs[hh][0:DK + 1, :],
                                    pvw_sb[:, sk, 2 * p + hh, :],
                                    pt[:, j, :],
                                    start=(grp == 0 and j == 0),
                                    stop=(grp == ST // 2 - 1 and j == 1))
                    for hh in range(2):
                        rcp = rcps.tile([1, CW], F32R, tag="rcp")
                        nc.vector.reciprocal(rcp[:], xaugs[hh][DK:DK + 1, :])
                        rep = misc_ps.tile([P, CW], F32, tag="m")
                        nc.tensor.matmul(rep[0:DK, :], ones1[:1, 0:DK],
                                         rcp[:], start=True, stop=True)
                        repc = repsb.tile([DK, CW], BF16, tag="repc")
                        nc.vector.tensor_copy(repc[:], rep[0:DK, :])
                        xsl = xn_sb[64 * hh:64 * hh + 64, p, csl]
                        nc.vector.tensor_tensor(
                            xsl, xaugs[hh][0:DK, :], repc[:], ALU.mult)
                        nc.vector.tensor_scalar_add(
                            xsl, xsl,
                            bv_sb[64 * hh:64 * hh + 64, p:p + 1])

                # ---- out-proj for this chunk (both p halves now ready) ----
                for st4 in range(4):
                    st = 4 * c + st4
                    for oc in range(2):
                        yp = misc_ps.tile([P, CW], F32, tag="m")
                        for p2 in range(2):
                            nc.tensor.matmul(yp[:], xn_sb[:, p2, ts(st, P)],
                                             wo_sb[:, p2, ds(CW * oc, CW)],
                                             start=(p2 == 0), stop=(p2 == 1))
                        ysb = ev.tile([P, CW], F32, tag="ysb")
                        nc.vector.tensor_tensor(ysb[:], yp[:],
                                                borep_sb[:, ds(CW * oc, CW)],
                                                ALU.add)
                        nc.sync.dma_start(y_r[st, :, ds(CW * oc, CW)], ysb[:])

    if apply_waitfix:
        _split_waits(nc, cap=1)
    return nc


_program_cache = {}


def get_program():
    if "nc" not in _program_cache:
        _program_cache["nc"] = build_program()
    return _program_cache["nc"]


def make_in_maps(Q, K, V, Wq, bq, Wk, bk, Wv, bv, Wo, bo):
    Q = np.asarray(Q, dtype=np.float32)
    K = np.asarray(K, dtype=np.float32)
    V = np.asarray(V, dtype=np.float32)
    Wq = np.asarray(Wq, dtype=np.float32)
    Wk = np.asarray(Wk, dtype=np.float32)
    Wv = np.asarray(Wv, dtype=np.float32)
    Wo = np.asarray(Wo, dtype=np.float32)
    bq = np.asarray(bq, dtype=np.float32)
    bk = np.asarray(bk, dtype=np.float32)
    bv = np.asarray(bv, dtype=np.float32)
    bo = np.asarray(bo, dtype=np.float32)

    def bf(a):
        return np.ascontiguousarray(a).astype(BF_NP)

    xt = {b: {
        "q": bf(Q[b].T),
        "k": bf(K[b].T),
        "v": bf(V[b].T),
    } for b in range(B)}

    ones1 = np.ones((1, P), dtype=np.float32)
    zero_bo = np.zeros((1, D), dtype=np.float32)
    bo_row = bo.reshape(1, D)

    in_maps = []
    for c in range(NCORES):
        b, g = divmod(c, GROUPS)
        hs = slice(DH * g, DH * (g + 1))
        in_maps.append({
            "xqt": xt[b]["q"],
            "xkt": xt[b]["k"],
            "xvt": xt[b]["v"],
            "wqt": bf(Wq[hs, :].T),
            "wkt": bf(Wk[hs, :].T),
            "wvt": bf(Wv[hs, :].T),
            "wot": bf(Wo[:, hs].T),
            "bq2": np.ascontiguousarray(bq[hs].reshape(2, P).T),
            "bk2": np.ascontiguousarray(bk[hs].reshape(2, P).T),
            "bv2": np.ascontiguousarray(bv[hs].reshape(2, P).T),
            "bo_eff": bo_row if g == 0 else zero_bo,
            "onesd": ones1,
        })
    return in_maps


def combine_outputs(results):
    """results: list of 8 per-core dicts with 'y' [S, D] partials."""
    out = np.zeros((B, S, D), dtype=np.float32)
    for c, res in enumerate(results):
        b = c // GROUPS
        out[b] += res["y"]
    return out


def kernel(**inputs) -> np.ndarray:
    nc = get_program()
    in_maps = make_in_maps(**inputs)
    res = run_bass_kernel_spmd(nc, in_maps, core_ids=list(range(NCORES)))
    return combine_outputs(res.results)


# revision 24
# speedup vs baseline: 1.6727x; 1.4223x over previous
"""Multi-head attention (nn_GroupQueryAttention_163208757512) on 8 TRN2 cores.

Problem: B=2, S=2048, D=1024, H=16 heads, DK=64. f32 in/out.
    q = Q @ Wq.T + bq  (per head)   k, v likewise
    out = softmax(q k^T / 8) v -> concat heads -> @ Wo.T + bo

Sharding: core c handles batch b=c//4 and head group g=c%4 (4 heads,
feature slice hs = 256*g : 256*g+256). Data parallel on B, tensor
parallel on heads; the output projection yields per-core partials that
the host sums (replaces the all-reduce).

Device-side design (v3 — bf16 matmuls, ScalarE-exp-bound inner loop):
  - host supplies X^T = {Q,K,V}[b].T pre-cast to bf16; weights are
    pre-arranged so every DMA row is a 2-4KB contiguous line.
  - projections: q^T,k^T in [dh, s] via W-stationary; v in [s, dh] via
    x-stationary; bv folded in as a rank-1 (ones x bv) matmul appended
    to each v accumulation group.
  - scores pre-transposed: S^T[sk, sq] = K_h Q_h^T, two heads
    row-packed via tile_position (K=64 each); exp on ScalarE from
    2-bank PSUM with scale=1/8, bf16 out. The score matmuls for sk+1
    are issued BEFORE the PV matmuls of sk (software skew) so the
    in-order PE queue never blocks ScalarE: phase C runs at ScalarE
    throughput.
  - PV is p-stationary: lhsT = pt[sk, sq-tile], rhs = [v_h | 1]
    (ones column accumulates denominators), out xaug[sq, 65] — the
    denominator is PER-PARTITION, so 1/denom is a [P,4] VectorE
    reciprocal and the normalize is one tensor_scalar_mul per tile
    (no cross-partition broadcast needed).
  - x_norm [sq, dh] is transposed back to [dh, sq] for the output
    projection with PE transpose-mode (identity operand), 128x128
    blocks.
  - output projection per chunk is interleaved into phase C; bo enters
    as a rank-1 (ones x bo) f32r matmul appended to each y
    accumulation group (only on the g==0 core of each batch).

Constraint discovered on this toolchain: walrus allows ONE sync-wait per
instruction, so a post-pass (split_waits) chains excess waits onto NoOps.
Custom DVE ops and ALU-divide are rejected by this walrus build.
Accumulation groups must keep one lhsT base partition (HW fault otherwise).
"""

import numpy as np
from contextlib import ExitStack

import ml_dtypes

import concourse.bass as bass
import concourse.mybir as mybir
import concourse.tile as tile
from concourse.bass import ds, ts
from concourse.bass_utils import run_bass_kernel_spmd

F32 = mybir.dt.float32
F32R = mybir.dt.float32r
BF16 = mybir.dt.bfloat16
AF = mybir.ActivationFunctionType
ALU = mybir.AluOpType
BF_NP = ml_dtypes.bfloat16

B, S, D, H = 2, 2048, 1024, 16
DK = D // H            # 64
NCORES = 8
GROUPS = 4             # head groups per batch
DH = D // GROUPS       # 256 feature cols per core
P = 128
KT = D // P            # 8 contraction tiles for projections
ST = S // P            # 16 s-tiles
CH = 4                 # s-chunks
CW = S // CH           # 512
SW = 2 * CW            # 1024-wide DMA staging (2KB bf16 lines)
HALVES = S // SW       # 2


# ---------------------------------------------------------------- wait fix
_wf_counter = [0]


def _split_waits(nc, cap=1):
    """walrus in this container accepts at most one sync-wait command per
    instruction; chain the rest onto same-engine NoOps placed just before."""
    for fn in nc.m.functions:
        for bb in fn.blocks:
            out, changed = [], False
            for inst in bb.instructions:
                si = inst.sync_info
                waits = list(si.on_wait) if (si is not None and si.on_wait) else []
                if len(waits) > cap:
                    changed = True
                    keep = waits[-cap:]
                    for i in range(0, len(waits) - cap, cap):
                        _wf_counter[0] += 1
                        out.append(mybir.InstNoOp(
                            name=f"waitfix_{_wf_counter[0]}",
                            sync_info=mybir.SyncInfo(
                                on_wait=waits[i:i + cap], on_update=[]),
                            engine=inst.engine,
                            bass_nofuse=True,
                        ))
                    inst.sync_info = mybir.SyncInfo(
                        on_wait=keep,
                        on_update=list(si.on_update) if si else [])
                out.append(inst)
            if changed:
                bb.instructions = out
    return nc


# ---------------------------------------------------------------- program
def build_program(apply_waitfix=True):
    nc = bass.Bass()

    xqt = nc.dram_tensor("xqt", [D, S], BF16, kind="ExternalInput")
    xkt = nc.dram_tensor("xkt", [D, S], BF16, kind="ExternalInput")
    xvt = nc.dram_tensor("xvt", [D, S], BF16, kind="ExternalInput")
    wq_h = nc.dram_tensor("wq_h", [P, KT * DH], BF16, kind="ExternalInput")
    wk_h = nc.dram_tensor("wk_h", [P, KT * DH], BF16, kind="ExternalInput")
    wv_h = nc.dram_tensor("wv_h", [P, KT * DH], BF16, kind="ExternalInput")
    wo_h = nc.dram_tensor("wo_h", [P, 2 * D], BF16, kind="ExternalInput")
    bq2 = nc.dram_tensor("bq2", [P, 2], F32, kind="ExternalInput")
    bk2 = nc.dram_tensor("bk2", [P, 2], F32, kind="ExternalInput")
    bvr = nc.dram_tensor("bvr", [1, DH], F32R, kind="ExternalInput")
    bo_eff = nc.dram_tensor("bo_eff", [1, D], F32R, kind="ExternalInput")
    onesd = nc.dram_tensor("onesd", [1, P], F32R, kind="ExternalInput")
    identd = nc.dram_tensor("identd", [P, P], BF16, kind="ExternalInput")
    y = nc.dram_tensor("y", [S, D], F32, kind="ExternalOutput")

    xqt_r = xqt.rearrange("(kt p) s -> kt p s", p=P)
    xkt_r = xkt.rearrange("(kt p) s -> kt p s", p=P)
    xvt_r = xvt.rearrange("(kt p) s -> kt p s", p=P)
    y_r = y.rearrange("(st p) d -> st p d", p=P)

    with tile.TileContext(nc) as tc:
      with ExitStack() as ctx:
        # ---- persistent SBUF ----
        wp = ctx.enter_context(tc.tile_pool(name="wp", bufs=1))
        wq_sb = wp.tile([P, KT, DH], BF16, tag="wq")
        wk_sb = wp.tile([P, KT, DH], BF16, tag="wk")
        wv_sb = wp.tile([P, KT, DH], BF16, tag="wv")
        wo_sb = wp.tile([P, 2, D], BF16, tag="wo")
        bq_sb = wp.tile([P, 2], F32, tag="bq")
        bk_sb = wp.tile([P, 2], F32, tag="bk")
        bvr_sb = wp.tile([1, DH], F32R, tag="bvr")
        ones1 = wp.tile([1, P], F32R, tag="ones1")
        bo_sb = wp.tile([1, D], F32R, tag="bo")
        ident = wp.tile([P, P], BF16, tag="ident")

        qt_sb = wp.tile([P, 2, S], BF16, tag="qt")
        kt_sb = wp.tile([P, 2, S], BF16, tag="kt")
        pvw_sb = wp.tile([P, ST, GROUPS, DK + 1], BF16, tag="pvw")
        xn_sb = wp.tile([P, 2, S], BF16, tag="xn")

        nc.sync.dma_start(wq_sb[:], wq_h.rearrange("p (kt m) -> p kt m", kt=KT))
        nc.sync.dma_start(wk_sb[:], wk_h.rearrange("p (kt m) -> p kt m", kt=KT))
        nc.sync.dma_start(wv_sb[:], wv_h.rearrange("p (kt m) -> p kt m", kt=KT))
        nc.sync.dma_start(bq_sb[:], bq2[:])
        nc.sync.dma_start(bk_sb[:], bk2[:])
        nc.sync.dma_start(bvr_sb[:], bvr[:])
        nc.sync.dma_start(ones1[:], onesd[:])
        # needed only from the first epilogue (~40us in)
        nc.sync.dma_start(wo_sb[:], wo_h.rearrange("p (p2 d) -> p p2 d", p2=2))
        nc.sync.dma_start(bo_sb[:], bo_eff[:])
        nc.sync.dma_start(ident[:], identd[:])
        # ones column of the [v | 1] PV stationary
        nc.vector.memset(pvw_sb[:, :, :, DK:DK + 1], 1.0)

        with nc.allow_low_precision(reason="bf16 matmuls, tol is 2e-2"):
          # ---------------- phase B: projections ----------------
          with (
              tc.tile_pool(name="xs", bufs=12) as xs,
              tc.tile_pool(name="qk", bufs=4, space="PSUM") as qk_ps,
              tc.tile_pool(name="vp", bufs=2, space="PSUM") as v_ps,
          ):
            for half in range(HALVES):
                wsl = ds(SW * half, SW)
                qx, kx, vx = [], [], []
                for kt in range(KT):
                    tq = xs.tile([P, SW], BF16, tag="xq")
                    nc.sync.dma_start(tq[:], xqt_r[kt, :, wsl])
                    qx.append(tq)
                    tk = xs.tile([P, SW], BF16, tag="xk")
                    nc.sync.dma_start(tk[:], xkt_r[kt, :, wsl])
                    kx.append(tk)
                    tv = xs.tile([P, SW], BF16, tag="xv")
                    nc.sync.dma_start(tv[:], xvt_r[kt, :, wsl])
                    vx.append(tv)

                for sub in range(2):
                    c = 2 * half + sub
                    csl = ds(CW * c, CW)
                    ssl = ds(CW * sub, CW)
                    ppq = [qk_ps.tile([P, CW], F32, tag="qk",
                                      name=f"ppq_{c}_{i}")
                           for i in range(2)]
                    ppk = [qk_ps.tile([P, CW], F32, tag="qk",
                                      name=f"ppk_{c}_{i}")
                           for i in range(2)]
                    vp = v_ps.tile([P, 4, DH], F32, tag="v",
                                   name=f"vp_{c}")
                    vp4 = [vp[:, i, :] for i in range(4)]
                    for kt in range(KT):
                        fl, ll = kt == 0, kt == KT - 1
                        for p in range(2):
                            nc.tensor.matmul(ppq[p][:],
                                             wq_sb[:, kt, ds(P * p, P)],
                                             qx[kt][:, ssl],
                                             start=fl, stop=ll)
                        for p in range(2):
                            nc.tensor.matmul(ppk[p][:],
                                             wk_sb[:, kt, ds(P * p, P)],
                                             kx[kt][:, ssl],
                                             start=fl, stop=ll)
                        for st4 in range(4):
                            # start only on the first group per PSUM bank:
                            # a start=True matmul clears has_written for the
                            # WHOLE bank (st4 0,1 share bank 0; 2,3 bank 1)
                            nc.tensor.matmul(
                                vp4[st4][:],
                                vx[kt][:, ds(CW * sub + P * st4, P)],
                                wv_sb[:, kt, :],
                                start=(fl and st4 % 2 == 0), stop=False)
                    for st4 in range(4):
                        # bv as rank-1 closes each v accumulation group
                        nc.tensor.matmul(vp4[st4][:], ones1[:1, 0:P],
                                         bvr_sb[:], start=False, stop=True)
                    for p in range(2):
                        nc.vector.tensor_scalar_add(qt_sb[:, p, csl],
                                                    ppq[p][:],
                                                    bq_sb[:, p:p + 1])
                        nc.vector.tensor_scalar_add(kt_sb[:, p, csl],
                                                    ppk[p][:],
                                                    bk_sb[:, p:p + 1])
                    for st4 in range(4):
                        nc.vector.tensor_copy(
                            pvw_sb[:, 4 * c + st4, :, 0:DK],
                            vp4[st4][:].rearrange("p (h d) -> p h d",
                                                  h=GROUPS))

          # ---------- phase C: attention + interleaved out-proj ----------
          with (
              tc.tile_pool(name="ptp", bufs=4) as ptp,
              tc.tile_pool(name="stgp", bufs=4) as stgp,
              tc.tile_pool(name="rcp", bufs=4) as rcpp,
              tc.tile_pool(name="ev", bufs=4) as ev,
              tc.tile_pool(name="spp", bufs=2, space="PSUM") as sp_ps,
              tc.tile_pool(name="xap", bufs=2, space="PSUM") as xa_ps,
              tc.tile_pool(name="mpp", bufs=2, space="PSUM") as misc_ps,
          ):
            for c in range(CH):
                csl = ds(CW * c, CW)
                for p in range(2):
                    xaugs = [xa_ps.tile([P, 4, DK + 1], F32, tag="xa",
                                        name=f"xa_{c}_{p}_{i}")
                             for i in range(2)]
                    pts = {}

                    def emit_pv(sk):
                        pt = pts.pop(sk)
                        for hh in range(2):
                            for m in range(4):
                                # xaug[hh] is one PSUM bank shared by the 4
                                # m-groups: start (bank-wide clear) only on
                                # the first matmul touching the bank
                                nc.tensor.matmul(
                                    xaugs[hh][:, m, :],
                                    pt[:, hh, ds(P * m, P)],
                                    pvw_sb[:, sk, 2 * p + hh, :],
                                    start=(sk == 0 and m == 0),
                                    stop=(sk == ST - 1))

                    for sk in range(ST):
                        sp = sp_ps.tile([P, 2, CW], F32, tag="sp")
                        for hh in range(2):
                            nc.tensor.matmul(
                                sp[:, hh, :],
                                kt_sb[64 * hh:64 * hh + 64, p, ts(sk, P)],
                                qt_sb[64 * hh:64 * hh + 64, p, csl],
                                start=True, stop=True,
                                tile_position=(64 * hh, 0))
                        pt = ptp.tile([P, 2, CW], BF16, tag="pt")
                        pts[sk] = pt
                        nc.scalar.activation(pt[:], sp[:], AF.Exp,
                                             scale=0.125)
                        if sk >= 1:
                            emit_pv(sk - 1)
                    emit_pv(ST - 1)

                    # ---- normalize + transpose back to [dh, s] ----
                    for hh in range(2):
                        rc = rcpp.tile([P, 4], F32, tag="rc")
                        nc.vector.reciprocal(rc[:],
                                             xaugs[hh][:, :, DK:DK + 1])
                        stg = stgp.tile([P, 2, P], BF16, tag="stg")
                        for m in range(4):
                            nc.vector.tensor_scalar_mul(
                                stg[:, m // 2, ds(64 * (m % 2), 64)],
                                xaugs[hh][:, m, 0:DK],
                                rc[:, m:m + 1])
                        for i in range(2):
                            tp = misc_ps.tile([P, 2 * CW], BF16, tag="m")
                            nc.tensor.transpose(tp[:, 0:P], stg[:, i, :],
                                                ident[:])
                            for jj in range(2):
                                m = 2 * i + jj
                                nc.vector.tensor_copy(
                                    xn_sb[64 * hh:64 * hh + 64, p,
                                          ds(CW * c + P * m, P)],
                                    tp[ds(64 * jj, 64), 0:P])

                # ---- out-proj for this chunk (both p halves ready) ----
                for st4 in range(4):
                    st = 4 * c + st4
                    for oc in range(2):
                        yp = misc_ps.tile([P, CW], F32, tag="m")
                        for p2 in range(2):
                            nc.tensor.matmul(yp[:], xn_sb[:, p2, ts(st, P)],
                                             wo_sb[:, p2, ds(CW * oc, CW)],
                                             start=(p2 == 0), stop=False)
                        nc.tensor.matmul(yp[:], ones1[:1, 0:P],
                                         bo_sb[:, ds(CW * oc, CW)],
                                         start=False, stop=True)
                        ysb = ev.tile([P, CW], F32, tag="ysb")
                        nc.vector.tensor_copy(ysb[:], yp[:])
                        nc.sync.dma_start(y_r[st, :, ds(CW * oc, CW)],
                                          ysb[:])

    if apply_waitfix:
        _split_waits(nc, cap=1)
    return nc


_program_cache = {}


def get_program():
    if "nc" not in _program_cache:
        _program_cache["nc"] = build_program()
    return _program_cache["nc"]


def _warr(wT):
    """[D', M] -> [P, (D'/P)*M] with 4KB-contiguous per-partition rows."""
    dp, m = wT.shape
    kt = dp // P
    return np.ascontiguousarray(
        wT.reshape(kt, P, m).transpose(1, 0, 2).reshape(P, kt * m)
    ).astype(BF_NP)


def make_in_maps(Q, K, V, Wq, bq, Wk, bk, Wv, bv, Wo, bo):
    Q = np.asarray(Q, dtype=np.float32)
    K = np.asarray(K, dtype=np.float32)
    V = np.asarray(V, dtype=np.float32)
    Wq = np.asarray(Wq, dtype=np.float32)
    Wk = np.asarray(Wk, dtype=np.float32)
    Wv = np.asarray(Wv, dtype=np.float32)
    Wo = np.asarray(Wo, dtype=np.float32)
    bq = np.asarray(bq, dtype=np.float32)
    bk = np.asarray(bk, dtype=np.float32)
    bv = np.asarray(bv, dtype=np.float32)
    bo = np.asarray(bo, dtype=np.float32)

    def bf(a):
        return np.ascontiguousarray(a).astype(BF_NP)

    xt = {b: {
        "q": bf(Q[b].T),
        "k": bf(K[b].T),
        "v": bf(V[b].T),
    } for b in range(B)}

    ones1 = np.ones((1, P), dtype=np.float32)
    identm = np.eye(P, dtype=np.float32).astype(BF_NP)
    zero_bo = np.zeros((1, D), dtype=np.float32)
    bo_row = np.ascontiguousarray(bo.reshape(1, D))

    in_maps = []
    for c in range(NCORES):
        b, g = divmod(c, GROUPS)
        hs = slice(DH * g, DH * (g + 1))
        in_maps.append({
            "xqt": xt[b]["q"],
            "xkt": xt[b]["k"],
            "xvt": xt[b]["v"],
            "wq_h": _warr(Wq[hs, :].T),
            "wk_h": _warr(Wk[hs, :].T),
            "wv_h": _warr(Wv[hs, :].T),
            "wo_h": _warr(Wo[:, hs].T),
            "bq2": np.ascontiguousarray(bq[hs].reshape(2, P).T),
            "bk2": np.ascontiguousarray(bk[hs].reshape(2, P).T),
            "bvr": np.ascontiguousarray(bv[hs].reshape(1, DH)),
            "bo_eff": bo_row if g == 0 else zero_bo,
            "onesd": ones1,
            "identd": identm,
        })
    return in_maps


def combine_outputs(results):
    """results: list of 8 per-core dicts with 'y' [S, D] partials."""
    out = np.zeros((B, S, D), dtype=np.float32)
    for c, res in enumerate(results):
        b = c // GROUPS
        out[b] += res["y"]
    return out


def kernel(**inputs) -> np.ndarray:
    nc = get_program()
    in_maps = make_in_maps(**inputs)
    res = run_bass_kernel_spmd(nc, in_maps, core_ids=list(range(NCORES)))
    return combine_outputs(res.results)


# revision 25
# speedup vs baseline: 1.7952x; 1.0732x over previous
"""Multi-head attention (nn_GroupQueryAttention_163208757512) on 8 TRN2 cores.

Problem: B=2, S=2048, D=1024, H=16 heads, DK=64. f32 in/out.
    q = Q @ Wq.T + bq  (per head)   k, v likewise
    out = softmax(q k^T / 8) v -> concat heads -> @ Wo.T + bo

Sharding: core c handles batch b=c//4 and head group g=c%4 (4 heads,
feature slice hs = 256*g : 256*g+256). Data parallel on B, tensor
parallel on heads; the output projection yields per-core partials that
the host sums (replaces the all-reduce).

Device-side design (v4 — one ScalarE-exp-bound software pipeline):
  - the kernel is one long stream of (score -> exp -> PV) iterations,
    ScalarE-bound (~1.1us/exp tile); ALL other PE work (v/q projections
    for later chunks, x_norm transposes, output-projection tiles) is
    chopped into ~1us jobs and popped one-per-iteration from a FIFO so
    the in-order PE queue never starves ScalarE.
  - prologue projects only what the first pass needs: k (all), v+q of
    chunk 0. DMA stream order matches consumption (wk, kx+vx half 0,
    kx half 1, qx half 0, vx half 1, qx half 1, then wo/ident/bo).
  - all matmuls bf16 (1 cycle/row @ 2.4GHz, FWL); f32 PSUM.
  - scores pre-transposed: S^T[sk, sq] = K_h Q_h^T, two heads
    row-packed via tile_position; exp on ScalarE from 2-bank PSUM,
    scale=1/8, bf16 out, issued one sk-tile ahead of PV (skew).
  - PV is p-stationary: lhsT = pt[sk, sq-tile], rhs = [v_h | 1] (ones
    column accumulates softmax denominators) -> xaug[sq, 4, 65]; the
    denominator is per-partition so normalize is a [P,4] reciprocal +
    tensor_scalar_mul (no cross-partition broadcast).
  - x_norm [sq, dh] transposes back to [dh, s] via PE transpose-mode.
  - biases: bq/bk via per-partition tensor_scalar on evacuation; bv and
    bo as rank-1 (ones x bias) f32r matmuls appended to the v / y
    accumulation groups.
  - PSUM: one 3-pool layout for the whole program — sp 2x2 banks
    (scores, and borrowed by projection jobs), xaug 2x1, misc 2x1
    (transpose-out + y tiles). start=True clears has_written for the
    WHOLE bank, so bank-sharing accumulation groups only set start on
    the first group per bank.

Constraint discovered on this toolchain: walrus allows ONE sync-wait per
instruction, so a post-pass (split_waits) chains excess waits onto NoOps.
Custom DVE ops and ALU-divide are rejected by this walrus build.
"""

import numpy as np
from collections import deque
from contextlib import ExitStack

import ml_dtypes

import concourse.bass as bass
import concourse.mybir as mybir
import concourse.tile as tile
from concourse.bass import ds, ts
from concourse.bass_utils import run_bass_kernel_spmd

F32 = mybir.dt.float32
F32R = mybir.dt.float32r
BF16 = mybir.dt.bfloat16
AF = mybir.ActivationFunctionType
ALU = mybir.AluOpType
BF_NP = ml_dtypes.bfloat16

B, S, D, H = 2, 2048, 1024, 16
DK = D // H            # 64
NCORES = 8
GROUPS = 4             # head groups per batch
DH = D // GROUPS       # 256 feature cols per core
P = 128
KT = D // P            # 8 contraction tiles for projections
ST = S // P            # 16 s-tiles
CH = 4                 # s-chunks
CW = S // CH           # 512
SW = 2 * CW            # 1024-wide DMA staging (2KB bf16 lines)
HALVES = S // SW       # 2


# ---------------------------------------------------------------- wait fix
_wf_counter = [0]


def _split_waits(nc, cap=1):
    """walrus in this container accepts at most one sync-wait command per
    instruction; chain the rest onto same-engine NoOps placed just before."""
    for fn in nc.m.functions:
        for bb in fn.blocks:
            out, changed = [], False
            for inst in bb.instructions:
                si = inst.sync_info
                waits = list(si.on_wait) if (si is not None and si.on_wait) else []
                if len(waits) > cap:
                    changed = True
                    keep = waits[-cap:]
                    for i in range(0, len(waits) - cap, cap):
                        _wf_counter[0] += 1
                        out.append(mybir.InstNoOp(
                            name=f"waitfix_{_wf_counter[0]}",
                            sync_info=mybir.SyncInfo(
                                on_wait=waits[i:i + cap], on_update=[]),
                            engine=inst.engine,
                            bass_nofuse=True,
                        ))
                    inst.sync_info = mybir.SyncInfo(
                        on_wait=keep,
                        on_update=list(si.on_update) if si else [])
                out.append(inst)
            if changed:
                bb.instructions = out
    return nc


# ---------------------------------------------------------------- program
def build_program(apply_waitfix=True):
    nc = bass.Bass()

    xqt = nc.dram_tensor("xqt", [D, S], BF16, kind="ExternalInput")
    xkt = nc.dram_tensor("xkt", [D, S], BF16, kind="ExternalInput")
    xvt = nc.dram_tensor("xvt", [D, S], BF16, kind="ExternalInput")
    wq_h = nc.dram_tensor("wq_h", [P, KT * DH], BF16, kind="ExternalInput")
    wk_h = nc.dram_tensor("wk_h", [P, KT * DH], BF16, kind="ExternalInput")
    wv_h = nc.dram_tensor("wv_h", [P, KT * DH], BF16, kind="ExternalInput")
    wo_h = nc.dram_tensor("wo_h", [P, 2 * D], BF16, kind="ExternalInput")
    bq2 = nc.dram_tensor("bq2", [P, 2], F32, kind="ExternalInput")
    bk2 = nc.dram_tensor("bk2", [P, 2], F32, kind="ExternalInput")
    bvr = nc.dram_tensor("bvr", [1, DH], F32R, kind="ExternalInput")
    bo_eff = nc.dram_tensor("bo_eff", [1, D], F32R, kind="ExternalInput")
    onesd = nc.dram_tensor("onesd", [1, P], F32R, kind="ExternalInput")
    identd = nc.dram_tensor("identd", [P, P], BF16, kind="ExternalInput")
    y = nc.dram_tensor("y", [S, D], F32, kind="ExternalOutput")

    xqt_r = xqt.rearrange("(kt p) s -> kt p s", p=P)
    xkt_r = xkt.rearrange("(kt p) s -> kt p s", p=P)
    xvt_r = xvt.rearrange("(kt p) s -> kt p s", p=P)
    y_r = y.rearrange("(st p) d -> st p d", p=P)

    with tile.TileContext(nc) as tc:
      with ExitStack() as ctx:
        # ---- persistent SBUF ----
        wp = ctx.enter_context(tc.tile_pool(name="wp", bufs=1))
        wq_sb = wp.tile([P, KT, DH], BF16, tag="wq")
        wk_sb = wp.tile([P, KT, DH], BF16, tag="wk")
        wv_sb = wp.tile([P, KT, DH], BF16, tag="wv")
        wo_sb = wp.tile([P, 2, D], BF16, tag="wo")
        bq_sb = wp.tile([P, 2], F32, tag="bq")
        bk_sb = wp.tile([P, 2], F32, tag="bk")
        bvr_sb = wp.tile([1, DH], F32R, tag="bvr")
        ones1 = wp.tile([1, P], F32R, tag="ones1")
        bo_sb = wp.tile([1, D], F32R, tag="bo")
        ident = wp.tile([P, P], BF16, tag="ident")

        qt_sb = wp.tile([P, 2, S], BF16, tag="qt")
        kt_sb = wp.tile([P, 2, S], BF16, tag="kt")
        pvw_sb = wp.tile([P, ST, GROUPS, DK + 1], BF16, tag="pvw")
        xn_sb = wp.tile([P, 2, S], BF16, tag="xn")

        # critical-path weight/bias DMAs (wo/ident/bo queued after the
        # x streams below — not needed until the first epilogue)
        nc.sync.dma_start(wk_sb[:], wk_h.rearrange("p (kt m) -> p kt m", kt=KT))
        nc.sync.dma_start(wv_sb[:], wv_h.rearrange("p (kt m) -> p kt m", kt=KT))
        nc.sync.dma_start(wq_sb[:], wq_h.rearrange("p (kt m) -> p kt m", kt=KT))
        nc.sync.dma_start(bq_sb[:], bq2[:])
        nc.sync.dma_start(bk_sb[:], bk2[:])
        nc.sync.dma_start(bvr_sb[:], bvr[:])
        nc.sync.dma_start(ones1[:], onesd[:])
        nc.vector.memset(pvw_sb[:, :, :, DK:DK + 1], 1.0)

        with nc.allow_low_precision(reason="bf16 matmuls, tol is 2e-2"):
          with (
              tc.tile_pool(name="xs", bufs=16) as xs,
              tc.tile_pool(name="ptp", bufs=4) as ptp,
              tc.tile_pool(name="stgp", bufs=4) as stgp,
              tc.tile_pool(name="rcp", bufs=4) as rcpp,
              tc.tile_pool(name="ev", bufs=4) as ev,
              tc.tile_pool(name="spp", bufs=2, space="PSUM") as sp_ps,
              tc.tile_pool(name="xap", bufs=2, space="PSUM") as xa_ps,
              tc.tile_pool(name="mpp", bufs=2, space="PSUM") as misc_ps,
          ):
            # ---- x staging: DMA stream in consumption order ----
            kx = [None] * KT * HALVES
            vx = [None] * KT * HALVES
            qx = [None] * KT * HALVES

            def xdma(dst, src_r, idx):
                half, kt = divmod(idx, KT)
                t = xs.tile([P, SW], BF16, tag=dst, name=f"{dst}_{idx}")
                nc.sync.dma_start(t[:], src_r[kt, :, ds(SW * half, SW)])
                return t

            for kt in range(KT):                      # k+v half 0
                kx[kt] = xdma("xk", xkt_r, kt)
                vx[kt] = xdma("xv", xvt_r, kt)
            for kt in range(KT):                      # k half 1
                kx[KT + kt] = xdma("xk", xkt_r, KT + kt)
            for kt in range(KT):                      # q half 0
                qx[kt] = xdma("xq", xqt_r, kt)
            for kt in range(KT):                      # v half 1
                vx[KT + kt] = xdma("xv", xvt_r, KT + kt)
            for kt in range(KT):                      # q half 1
                qx[KT + kt] = xdma("xq", xqt_r, KT + kt)
            nc.sync.dma_start(wo_sb[:],
                              wo_h.rearrange("p (p2 d) -> p p2 d", p2=2))
            nc.sync.dma_start(bo_sb[:], bo_eff[:])
            nc.sync.dma_start(ident[:], identd[:])

            # ---------------- projection emitters ----------------
            def kq_proj(c, which):
                """project q or k for chunk c into {q,k}t_sb[:, :, csl]."""
                half, sub = divmod(c, 2)
                ssl = ds(CW * sub, CW)
                w_sb, x_t, out_sb, b_sb = (
                    (wq_sb, qx, qt_sb, bq_sb) if which == "q"
                    else (wk_sb, kx, kt_sb, bk_sb))
                pp = sp_ps.tile([P, 2, CW], F32, tag="sp",
                                name=f"pp{which}_{c}")
                for kt in range(KT):
                    for p in range(2):
                        nc.tensor.matmul(pp[:, p, :],
                                         w_sb[:, kt, ds(P * p, P)],
                                         x_t[KT * half + kt][:, ssl],
                                         start=(kt == 0),
                                         stop=(kt == KT - 1))
                for p in range(2):
                    nc.vector.tensor_scalar_add(out_sb[:, p, ds(CW * c, CW)],
                                                pp[:, p, :],
                                                b_sb[:, p:p + 1])

            def v_proj(c):
                """project v for chunk c into pvw_sb[:, 4c:4c+4, :, 0:DK]."""
                half, sub = divmod(c, 2)
                vp = sp_ps.tile([P, 2, CW], F32, tag="sp", name=f"ppv_{c}")
                for kt in range(KT):
                    for st4 in range(4):
                        j, u = divmod(st4, 2)
                        nc.tensor.matmul(
                            vp[:, j, ds(DH * u, DH)],
                            vx[KT * half + kt][:, ds(CW * sub + P * st4, P)],
                            wv_sb[:, kt, :],
                            start=(kt == 0 and u == 0), stop=False)
                for st4 in range(4):
                    j, u = divmod(st4, 2)
                    nc.tensor.matmul(vp[:, j, ds(DH * u, DH)],
                                     ones1[:1, 0:P], bvr_sb[:],
                                     start=False, stop=True)
                for st4 in range(4):
                    j, u = divmod(st4, 2)
                    nc.vector.tensor_copy(
                        pvw_sb[:, 4 * c + st4, :, 0:DK],
                        vp[:, j, ds(DH * u, DH)].rearrange(
                            "p (h d) -> p h d", h=GROUPS))

            # ---------------- deferred-job emitters ----------------
            def transpose_job(c, p, hh, i, stg):
                def run():
                    tp = misc_ps.tile([P, 2 * CW], BF16, tag="m",
                                      name=f"tp_{c}_{p}_{hh}_{i}")
                    nc.tensor.transpose(tp[:, 0:P], stg[:, i, :], ident[:])
                    for jj in range(2):
                        m = 2 * i + jj
                        nc.vector.tensor_copy(
                            xn_sb[64 * hh:64 * hh + 64, p,
                                  ds(CW * c + P * m, P)],
                            tp[ds(64 * jj, 64), 0:P])
                return run

            def ytile_job(st, oc):
                def run():
                    yp = misc_ps.tile([P, CW], F32, tag="m",
                                      name=f"yp_{st}_{oc}")
                    for p2 in range(2):
                        nc.tensor.matmul(yp[:], xn_sb[:, p2, ts(st, P)],
                                         wo_sb[:, p2, ds(CW * oc, CW)],
                                         start=(p2 == 0), stop=False)
                    nc.tensor.matmul(yp[:], ones1[:1, 0:P],
                                     bo_sb[:, ds(CW * oc, CW)],
                                     start=False, stop=True)
                    ysb = ev.tile([P, CW], F32, tag="ysb",
                                  name=f"ysb_{st}_{oc}")
                    nc.vector.tensor_copy(ysb[:], yp[:])
                    nc.sync.dma_start(y_r[st, :, ds(CW * oc, CW)], ysb[:])
                return run

            # ---------------- prologue ----------------
            for c in range(CH):
                kq_proj(c, "k")
            v_proj(0)
            kq_proj(0, "q")

            pending = deque()

            # ---------------- the ScalarE-bound pipeline ----------------
            for t in range(2 * CH):
                c, p = divmod(t, 2)
                csl = ds(CW * c, CW)
                if t == 0:
                    pending.append(lambda: v_proj(1))
                    pending.append(lambda: v_proj(2))
                    pending.append(lambda: v_proj(3))
                if p == 0 and c + 1 < CH:
                    cc = c + 1
                    pending.append(lambda cc=cc: kq_proj(cc, "q"))

                xaugs = [xa_ps.tile([P, 4, DK + 1], F32, tag="xa",
                                    name=f"xa_{c}_{p}_{i}")
                         for i in range(2)]
                pts = {}

                def emit_pv(sk, c=c, p=p, xaugs=xaugs, pts=pts):
                    pt = pts.pop(sk)
                    for hh in range(2):
                        for m in range(4):
                            # xaug[hh] = one bank shared by 4 m-groups:
                            # bank-wide clear only on the first matmul
                            nc.tensor.matmul(
                                xaugs[hh][:, m, :],
                                pt[:, hh, ds(P * m, P)],
                                pvw_sb[:, sk, 2 * p + hh, :],
                                start=(sk == 0 and m == 0),
                                stop=(sk == ST - 1))

                for sk in range(ST):
                    sp = sp_ps.tile([P, 2, CW], F32, tag="sp",
                                    name=f"sp_{t}_{sk}")
                    for hh in range(2):
                        nc.tensor.matmul(
                            sp[:, hh, :],
                            kt_sb[64 * hh:64 * hh + 64, p, ts(sk, P)],
                            qt_sb[64 * hh:64 * hh + 64, p, csl],
                            start=True, stop=True,
                            tile_position=(64 * hh, 0))
                    pt = ptp.tile([P, 2, CW], BF16, tag="pt",
                                  name=f"pt_{t}_{sk}")
                    pts[sk] = pt
                    nc.scalar.activation(pt[:], sp[:], AF.Exp, scale=0.125)
                    if sk >= 1:
                        emit_pv(sk - 1)
                    if pending:
                        pending.popleft()()
                emit_pv(ST - 1)

                # ---- normalize; transposes are deferred jobs ----
                for hh in range(2):
                    rc = rcpp.tile([P, 4], F32, tag="rc",
                                   name=f"rc_{c}_{p}_{hh}")
                    nc.vector.reciprocal(rc[:], xaugs[hh][:, :, DK:DK + 1])
                    stg = stgp.tile([P, 2, P], BF16, tag="stg",
                                    name=f"stg_{c}_{p}_{hh}")
                    for m in range(4):
                        nc.vector.tensor_scalar_mul(
                            stg[:, m // 2, ds(64 * (m % 2), 64)],
                            xaugs[hh][:, m, 0:DK],
                            rc[:, m:m + 1])
                    for i in range(2):
                        pending.append(transpose_job(c, p, hh, i, stg))
                if p == 1:
                    for st4 in range(4):
                        for oc in range(2):
                            pending.append(ytile_job(4 * c + st4, oc))

            while pending:
                pending.popleft()()

    if apply_waitfix:
        _split_waits(nc, cap=1)
    return nc


_program_cache = {}


def get_program():
    if "nc" not in _program_cache:
        _program_cache["nc"] = build_program()
    return _program_cache["nc"]


def _warr(wT):
    """[D', M] -> [P, (D'/P)*M] with 4KB-contiguous per-partition rows."""
    dp, m = wT.shape
    kt = dp // P
    return np.ascontiguousarray(
        wT.reshape(kt, P, m).transpose(1, 0, 2).reshape(P, kt * m)
    ).astype(BF_NP)


def make_in_maps(Q, K, V, Wq, bq, Wk, bk, Wv, bv, Wo, bo):
    Q = np.asarray(Q, dtype=np.float32)
    K = np.asarray(K, dtype=np.float32)
    V = np.asarray(V, dtype=np.float32)
    Wq = np.asarray(Wq, dtype=np.float32)
    Wk = np.asarray(Wk, dtype=np.float32)
    Wv = np.asarray(Wv, dtype=np.float32)
    Wo = np.asarray(Wo, dtype=np.float32)
    bq = np.asarray(bq, dtype=np.float32)
    bk = np.asarray(bk, dtype=np.float32)
    bv = np.asarray(bv, dtype=np.float32)
    bo = np.asarray(bo, dtype=np.float32)

    def bf(a):
        return np.ascontiguousarray(a).astype(BF_NP)

    xt = {b: {
        "q": bf(Q[b].T),
        "k": bf(K[b].T),
        "v": bf(V[b].T),
    } for b in range(B)}

    ones1 = np.ones((1, P), dtype=np.float32)
    identm = np.eye(P, dtype=np.float32).astype(BF_NP)
    zero_bo = np.zeros((1, D), dtype=np.float32)
    bo_row = np.ascontiguousarray(bo.reshape(1, D))

    in_maps = []
    for c in range(NCORES):
        b, g = divmod(c, GROUPS)
        hs = slice(DH * g, DH * (g + 1))
        in_maps.append({
            "xqt": xt[b]["q"],
            "xkt": xt[b]["k"],
            "xvt": xt[b]["v"],
            "wq_h": _warr(Wq[hs, :].T),
            "wk_h": _warr(Wk[hs, :].T),
            "wv_h": _warr(Wv[hs, :].T),
            "wo_h": _warr(Wo[:, hs].T),
            "bq2": np.ascontiguousarray(bq[hs].reshape(2, P).T),
            "bk2": np.ascontiguousarray(bk[hs].reshape(2, P).T),
            "bvr": np.ascontiguousarray(bv[hs].reshape(1, DH)),
            "bo_eff": bo_row if g == 0 else zero_bo,
            "onesd": ones1,
            "identd": identm,
        })
    return in_maps


def combine_outputs(results):
    """results: list of 8 per-core dicts with 'y' [S, D] partials."""
    out = np.zeros((B, S, D), dtype=np.float32)
    for c, res in enumerate(results):
        b = c // GROUPS
        out[b] += res["y"]
    return out


def kernel(**inputs) -> np.ndarray:
    nc = get_program()
    in_maps = make_in_maps(**inputs)
    res = run_bass_kernel_spmd(nc, in_maps, core_ids=list(range(NCORES)))
    return combine_outputs(res.results)
